# revision 43
# baseline (speedup 1.0000x reference)
"""HANLayer (2x GATConv + semantic attention) Trainium2 Bass kernel, 8 cores.

Strategy v2: aggregate in x-space (512-wide) instead of h-space (2048-wide),
exploiting linearity of segment-sum: sum_e alpha_e (x[src_e] @ W) =
(sum_e alpha_e x[src_e]) @ W.  This removes the replicated full projection
(x @ [W1 W2], 21.5 GFLOP/core) of the v1 kernel entirely.

Per core: 10 dst-node blocks of 128; edges sorted by dst, chunked by 128.
Per-edge attention-logit scalars s=x@(W@a_src), d=x@(W@a_dst) are computed
once for all nodes ([NPAD, 4]), scattered to a 256B-row DRAM table, and
fetched per-edge with batched dma_gather (one instruction per block) along
with the x rows.  One-hot edge->dst matrices (host-built) turn the weighted
segment-sum into matmuls accumulating aggT = sum alpha_e x_e^T directly in
transposed orientation, so the post-aggregation projection needs no
transposes.  Semantic attention runs pipelined per block group.
"""
import os
import sys

for _p in ("/opt/trn_rl_repo", "/root/.axon_site/_ro/trn_rl_repo"):
    if os.path.isdir(_p) and _p not in sys.path:
        sys.path.insert(0, _p)

import numpy as np
import ml_dtypes

import concourse.bacc as bacc
import concourse.bass as bass
import concourse.mybir as mybir
import concourse.tile as tile
from concourse import bass_utils
from concourse.masks import make_identity

F32 = mybir.dt.float32
BF16 = mybir.dt.bfloat16
I32 = mybir.dt.int32

N = 10000
E = 160000
IN_C = 512
OUT_C = 1024
NEG_SLOPE = 0.2
NCORES = 8
NPAD = 10240
NBLK = NPAD // 128      # 80 dst blocks total
BPC = 10                # dst blocks per core
NODES_PER_CORE = 1280
P = 128

AddOp = mybir.AluOpType.add
SubOp = mybir.AluOpType.subtract
MulOp = mybir.AluOpType.mult
MaxOp = mybir.AluOpType.max


def _host_prep(edge_index):
    """Sort edges (plus self loops incl. pad nodes) by dst; build per-core
    per-chunk gather indices and one-hot edge<->dst matrices (both
    orientations) per 128-edge chunk."""
    src = np.concatenate([edge_index[0].astype(np.int64),
                          np.arange(NPAD, dtype=np.int64)])
    dst = np.concatenate([edge_index[1].astype(np.int64),
                          np.arange(NPAD, dtype=np.int64)])
    order = np.argsort(dst, kind="stable")
    src_s = src[order]
    dst_s = dst[order]
    blk = dst_s // P
    counts = np.bincount(blk, minlength=NBLK)
    K = int(np.ceil(counts.max() / P))
    CE = K * P
    src32 = np.zeros((NCORES, P, BPC * K), np.int32)
    em = np.zeros((NCORES, BPC, P, CE), np.float32)
    emt = np.zeros((NCORES, BPC, P, CE), np.float32)
    bstart = np.searchsorted(blk, np.arange(NBLK + 1))
    for b in range(NBLK):
        core, bslot = divmod(b, BPC)
        lo, hi = bstart[b], bstart[b + 1]
        nb = hi - lo
        fs = np.zeros(CE, np.int64)
        fs[:nb] = src_s[lo:hi]
        ii = np.arange(nb)
        dloc = dst_s[lo:hi] - b * P
        em[core, bslot, ii % P, (ii // P) * P + dloc] = 1.0
        emt[core, bslot, dloc, (ii // P) * P + ii % P] = 1.0
        src32[core, :, bslot * K:(bslot + 1) * K] = fs.reshape(K, P).T
    dbidx = np.zeros((NCORES, P, BPC), np.int32)
    for core in range(NCORES):
        for b in range(BPC):
            dbidx[core, :, b] = (core * BPC + b) * P + np.arange(P)
    return K, src32, em, emt, dbidx


def _build_program(K, debug=False):
    CL = K * P          # padded edges per block
    nc = bacc.Bacc("TRN2", target_bir_lowering=False, debug=False,
                   enable_asserts=False, num_devices=NCORES)

    RW = IN_C + 4       # XS row: [x(512) | s1 s2 | d1 d2]
    X516 = nc.dram_tensor("X516", [NPAD, RW], BF16, kind="ExternalInput")
    XT = nc.dram_tensor("XT", [IN_C, NPAD], BF16, kind="ExternalInput")
    WT12 = nc.dram_tensor("WT12", [2 * OUT_C, IN_C], BF16,
                          kind="ExternalInput")
    A4 = nc.dram_tensor("A4", [OUT_C, 4], BF16, kind="ExternalInput")
    W12 = nc.dram_tensor("W12", [2 * IN_C, OUT_C], BF16, kind="ExternalInput")
    WP1 = nc.dram_tensor("WP1", [OUT_C, OUT_C], BF16, kind="ExternalInput")
    WP2 = nc.dram_tensor("WP2", [OUT_C, OUT_C], BF16, kind="ExternalInput")
    B12 = nc.dram_tensor("B12", [2, OUT_C], F32, kind="ExternalInput")
    BP1C = nc.dram_tensor("BP1C", [P, 8], F32, kind="ExternalInput")
    PRA = nc.dram_tensor("PRA", [1, 1], F32, kind="ExternalInput")
    MSK = nc.dram_tensor("MSK", [1, 1], F32, kind="ExternalInput")
    EMB = nc.dram_tensor("EMB", [BPC, P, CL], BF16, kind="ExternalInput")
    EMTB = nc.dram_tensor("EMTB", [BPC, P, CL], BF16, kind="ExternalInput")
    SRC32 = nc.dram_tensor("SRC32", [P, BPC * K], I32, kind="ExternalInput")
    DBIDX = nc.dram_tensor("DBIDX", [P, BPC], I32, kind="ExternalInput")

    OUT = nc.dram_tensor("OUT", [NODES_PER_CORE, OUT_C], F32,
                         kind="ExternalOutput")

    # s/d scalars are scattered into the tail columns of X516 itself (the
    # runtime re-uploads inputs each call, so device-side mutation is safe)
    XS = X516
    ARIN = nc.dram_tensor("ARIN", [OUT_C], F32, kind="Internal")
    AROUT = nc.dram_tensor("AROUT", [OUT_C], F32, kind="Internal",
                           addr_space="Shared")
    ATTD = nc.dram_tensor("ATTD", [1, OUT_C], F32, kind="Internal")

    # block group boundaries for the semantic-attention (wp1/tanh) passes;
    # node 1040 within a core is where valid nodes end on the last core
    GROUPS = [(0, 4), (4, 8), (8, 10)]

    with tile.TileContext(nc) as tc:
        with tc.tile_pool(name="persist", bufs=1) as pp:
            b1b = pp.tile([P, OUT_C], F32, tag="b1b")
            b2b = pp.tile([P, OUT_C], F32, tag="b2b")
            nc.sync.dma_start(b1b[:], B12.ap()[0:1, :].to_broadcast((P, OUT_C)))
            nc.sync.dma_start(b2b[:], B12.ap()[1:2, :].to_broadcast((P, OUT_C)))
            bp1c = pp.tile([P, 8], F32, tag="bp1c")
            nc.sync.dma_start(bp1c[:], BP1C.ap())
            pa_col = pp.tile([P, 1], F32, tag="pa_col")
            nc.sync.dma_start(pa_col[:], PRA.ap().to_broadcast((P, 1)))
            msk_col = pp.tile([P, 1], F32, tag="msk_col")
            nc.sync.dma_start(msk_col[:], MSK.ap().to_broadcast((P, 1)))
            ones_bf = pp.tile([P, 1], BF16, tag="ones")
            nc.vector.memset(ones_bf[:], 1.0)
            ident = pp.tile([P, P], BF16, tag="ident")
            make_identity(nc, ident[:])
            sidx = pp.tile([P, BPC * K], I32, tag="sidx")
            nc.sync.dma_start(sidx[:], SRC32.ap())
            dbix = pp.tile([P, BPC], I32, tag="dbix")
            nc.sync.dma_start(dbix[:], DBIDX.ap())
            sAcc = pp.tile([P, 4 * NBLK], BF16, tag="sAcc")
            accT = pp.tile([P, 32], F32, tag="accT")
            tbar = pp.tile([P, 8], F32, tag="tbar")
            h1st = pp.tile([P, BPC * OUT_C], BF16, tag="h1st")
            h2st = pp.tile([P, BPC * OUT_C], BF16, tag="h2st")
            w12t = [pp.tile([P, OUT_C], BF16, tag=f"w12_{i}", name=f"w12_{i}")
                    for i in range(8)]
            wp1t = [pp.tile([P, OUT_C], BF16, tag=f"wp1_{k}", name=f"wp1_{k}")
                    for k in range(8)]

            # ============ Phase A: attention-logit scalars S256 ============
            with tc.tile_pool(name="pAsb", bufs=1) as spA, \
                 tc.tile_pool(name="pAps", bufs=1, space="PSUM") as psA:
                a4t = [spA.tile([P, 4], BF16, tag=f"a4_{k}", name=f"a4_{k}")
                       for k in range(8)]
                for k in range(8):
                    nc.sync.dma_start(a4t[k][:], A4.ap()[k * P:(k + 1) * P, :])
                wtt = [spA.tile([P, IN_C], BF16, tag=f"wt_{i}", name=f"wt_{i}")
                       for i in range(16)]
                for i in range(16):
                    nc.sync.dma_start(wtt[i][:],
                                      WT12.ap()[i * P:(i + 1) * P, :])
                # wtilde[:, 0:2] = [W1@a_src1, W1@a_dst1] (cols s1,d1)
                # wtilde[:, 2:4] = [W2@a_src2, W2@a_dst2] (cols s2,d2)
                w4sb = spA.tile([P, 16], BF16, tag="w4sb")
                for ic in range(4):
                    wps1 = psA.tile([P, 2], F32, tag="wps1", bufs=2)
                    wps2 = psA.tile([P, 2], F32, tag="wps2", bufs=2)
                    for k in range(8):
                        nc.tensor.matmul(wps1[:],
                                         lhsT=wtt[k][:, ic * P:(ic + 1) * P],
                                         rhs=a4t[k][:, 0:2],
                                         start=(k == 0), stop=(k == 7))
                    for k in range(8):
                        nc.tensor.matmul(wps2[:],
                                         lhsT=wtt[8 + k][:, ic * P:(ic + 1) * P],
                                         rhs=a4t[k][:, 2:4],
                                         start=(k == 0), stop=(k == 7))
                    nc.vector.tensor_copy(w4sb[:, ic * 4:ic * 4 + 2], wps1[:])
                    nc.vector.tensor_copy(w4sb[:, ic * 4 + 2:ic * 4 + 4],
                                          wps2[:])
                # XT loaded in two NPAD halves so S matmuls start early
                HB = NPAD // 2
                xtg = [[spA.tile([P, HB], BF16, tag=f"xtg{g}_{h}",
                                 name=f"xtg{g}_{h}")
                        for g in range(4)] for h in range(2)]
                for h in range(2):
                    for g in range(4):
                        nc.sync.dma_start(
                            xtg[h][g][:],
                            XT.ap()[g * P:(g + 1) * P, h * HB:(h + 1) * HB])
                # weight tiles for later phases load behind the critical XT
                for i in range(8):
                    nc.sync.dma_start(w12t[i][:],
                                      W12.ap()[i * P:(i + 1) * P, :])
                    nc.sync.dma_start(wp1t[i][:],
                                      WP1.ap()[i * P:(i + 1) * P, :])
                for i in range(NBLK):
                    h, ih = divmod(i, NBLK // 2)
                    sps = psA.tile([P, 4], F32, tag="sps", bufs=4)
                    for g in range(4):
                        nc.tensor.matmul(sps[:],
                                         lhsT=xtg[h][g][:, ih * P:
                                                        (ih + 1) * P],
                                         rhs=w4sb[:, g * 4:(g + 1) * 4],
                                         start=(g == 0), stop=(g == 3))
                    # psum cols [s1,d1,s2,d2] -> XS tail order [s1,s2,d1,d2]
                    nc.vector.tensor_copy(sAcc[:, 4 * i:4 * i + 2],
                                          sps[:, 0::2])
                    nc.vector.tensor_copy(sAcc[:, 4 * i + 2:4 * i + 4],
                                          sps[:, 1::2])
                nc.sync.dma_start(
                    bass.AP(XS, IN_C, [[RW, P], [RW * P, NBLK], [1, 4]]),
                    sAcc[:].rearrange("p (i c) -> p i c", i=NBLK))

            # ============ Phase B: aggregation + proj + semantic ============
            with tc.tile_pool(name="pBsb", bufs=1) as sp, \
                 tc.tile_pool(name="pBps", bufs=1, space="PSUM") as ps:
                aggT12 = ps.tile([P, 1024], F32, tag="agg")
                ph = ps.tile([P, OUT_C], F32, tag="ph")
                tpp = ps.tile([P, 8 * P], BF16, tag="tpp")
                pre = {}
                st = {}
                htgt = None

                def prep_io(b):
                    """Issue all DMAs/gathers for block b (one block ahead)."""
                    dblkr = sp.tile([P, RW], BF16, tag="dblkr", bufs=2)
                    nc.gpsimd.indirect_dma_start(
                        out=dblkr[:], out_offset=None, in_=XS.ap(),
                        in_offset=bass.IndirectOffsetOnAxis(
                            ap=dbix[:, b:b + 1], axis=0))
                    emb = sp.tile([P, CL], BF16, tag="emb", bufs=2)
                    nc.sync.dma_start(emb[:], EMB.ap()[b])
                    emtb = sp.tile([P, CL], BF16, tag="emtb", bufs=2)
                    nc.sync.dma_start(emtb[:], EMTB.ap()[b])
                    xg = sp.tile([P, K * RW], BF16, tag="xg", bufs=2)
                    for c in range(K):
                        nc.gpsimd.indirect_dma_start(
                            out=xg[:, c * RW:(c + 1) * RW], out_offset=None,
                            in_=XS.ap(),
                            in_offset=bass.IndirectOffsetOnAxis(
                                ap=sidx[:, b * K + c:b * K + c + 1], axis=0))
                    pre[("io", b)] = (dblkr, emb, emtb, xg)

                def prep_compute(b):
                    """d logits (PE) + alpha tiles (DVE/ACT) for block b."""
                    dblkr, emb, emtb, xg = pre.pop(("io", b))
                    dblk = sp.tile([P, 2], BF16, tag="dblk", bufs=2)
                    nc.vector.tensor_copy(dblk[:], dblkr[:, IN_C + 2:RW])
                    # dd bank: group 1 = per-edge d logits (cols 0:2K);
                    # reused later by group 2 = denominators (cols 0:2)
                    dd = ps.tile([P, 512], F32, tag="dd", bufs=2)
                    for c in range(K):
                        nc.tensor.matmul(dd[:, 2 * c:2 * c + 2],
                                         lhsT=emtb[:, c * P:(c + 1) * P],
                                         rhs=dblk[:],
                                         start=(c == 0), stop=(c == K - 1))
                    e2 = sp.tile([P, 2 * K], F32, tag="e2", bufs=2)
                    nc.vector.tensor_tensor(
                        out=e2[:].rearrange("p (k t) -> p k t", k=K),
                        in0=xg[:].rearrange("p (k e) -> p k e", k=K)[
                            :, :, IN_C:IN_C + 2],
                        in1=dd[:, 0:2 * K].rearrange("p (k t) -> p k t", k=K),
                        op=AddOp)
                    lr = sp.tile([P, 2 * K], F32, tag="lr", bufs=2)
                    nc.vector.scalar_tensor_tensor(
                        out=lr[:], in0=e2[:], scalar=NEG_SLOPE, in1=e2[:],
                        op0=MulOp, op1=MaxOp)
                    al = sp.tile([P, 2 * K], F32, tag="al", bufs=2)
                    nc.scalar.activation(al[:], lr[:],
                                         mybir.ActivationFunctionType.Exp)
                    alh = sp.tile([P, 2 * K], BF16, tag="alh", bufs=2)
                    nc.vector.tensor_copy(alh[:], al[:])
                    pre[b] = (xg, emb, None, alh, dd, al)

                def prep_alpha(b):
                    xg, emb, _, alh, dd, al = pre[b]
                    a12b = sp.tile([P, K * 256], BF16, tag="a12b", bufs=2)
                    for c in range(K):
                        nc.vector.tensor_scalar_mul(
                            a12b[:, c * 256:c * 256 + P],
                            emb[:, c * P:(c + 1) * P],
                            al[:, 2 * c:2 * c + 1])
                        nc.vector.tensor_scalar_mul(
                            a12b[:, c * 256 + P:(c + 1) * 256],
                            emb[:, c * P:(c + 1) * P],
                            al[:, 2 * c + 1:2 * c + 2])
                    pre[b] = (xg, emb, a12b, alh, dd, al)

                def agg_matmuls(b):
                    xg, emb, a12b, alh, dd, al = pre[b]
                    for c in range(K):
                        # start=True clears has_written for the WHOLE psum
                        # bank: exactly one start/stop per bank (j pairs
                        # share a 512-f32 bank)
                        for j in range(4):
                            nc.tensor.matmul(
                                aggT12[:, j * 256:(j + 1) * 256],
                                lhsT=xg[:, c * RW + j * P:
                                        c * RW + (j + 1) * P],
                                rhs=a12b[:, c * 256:(c + 1) * 256],
                                start=(c == 0 and j % 2 == 0),
                                stop=(c == K - 1 and j % 2 == 1))
                        # den[dst, l] += sum_e em[e,dst] * alpha[e, l]
                        # (second sequential group in the dd bank)
                        nc.tensor.matmul(dd[:, 0:2],
                                         lhsT=emb[:, c * P:(c + 1) * P],
                                         rhs=alh[:, 2 * c:2 * c + 2],
                                         start=(c == 0), stop=(c == K - 1))

                def agg_copies(b):
                    xg, emb, a12b, alh, dd, al = pre.pop(b)
                    a1sb = sp.tile([P, IN_C], BF16, tag="a1sb", bufs=2)
                    a2sb = sp.tile([P, IN_C], BF16, tag="a2sb", bufs=2)
                    for j in range(4):
                        nc.vector.tensor_copy(
                            a1sb[:, j * P:(j + 1) * P],
                            aggT12[:, j * 256:j * 256 + P])
                        nc.vector.tensor_copy(
                            a2sb[:, j * P:(j + 1) * P],
                            aggT12[:, j * 256 + P:(j + 1) * 256])
                    rden = sp.tile([P, 2], F32, tag="rden", bufs=2)
                    nc.vector.reciprocal(rden[:], dd[:, 0:2])
                    st[b] = (a1sb, a2sb, rden)

                def stage2a(b):
                    a1sb, a2sb, rden = st.pop(b)
                    hcols = slice(b * OUT_C, (b + 1) * OUT_C)
                    for l, (asb, bb, hst) in enumerate(
                            [(a1sb, b1b, h1st), (a2sb, b2b, h2st)]):
                        for j in range(4):
                            for hh in range(2):
                                nc.tensor.matmul(
                                    ph[:, hh * 512:(hh + 1) * 512],
                                    lhsT=asb[:, j * P:(j + 1) * P],
                                    rhs=w12t[l * 4 + j][:, hh * 512:
                                                        (hh + 1) * 512],
                                    start=(j == 0), stop=(j == 3))
                        hstage = sp.tile([P, OUT_C], BF16, tag="hstage",
                                         bufs=2)
                        nc.vector.scalar_tensor_tensor(
                            out=hstage[:], in0=ph[:],
                            scalar=rden[:, l:l + 1], in1=bb[:],
                            op0=MulOp, op1=AddOp)
                        # prelu(v) = max(a*v, v) for 0<=a<=1
                        nc.vector.scalar_tensor_tensor(
                            out=hst[:, hcols], in0=hstage[:],
                            scalar=pa_col[:, 0:1], in1=hstage[:],
                            op0=MulOp, op1=MaxOp)
                    hsum = sp.tile([P, OUT_C], BF16, tag="hsum", bufs=2)
                    nc.vector.tensor_tensor(out=hsum[:], in0=h1st[:, hcols],
                                            in1=h2st[:, hcols], op=AddOp)
                    st[("hsum", b)] = hsum

                def stage2b(b):
                    nonlocal htgt
                    hsum = st.pop(("hsum", b))
                    g = 0 if b < 4 else (1 if b < 8 else 2)
                    glo, ghi = GROUPS[g]
                    gw = (ghi - glo) * P
                    if b == glo:
                        htgt = [sp.tile([P, gw], BF16, tag=f"htg{q}", bufs=2,
                                        name=f"htg{q}_{g}")
                                for q in range(8)]
                    bi = b - glo
                    # 8 transposes packed into one bf16 psum bank: single
                    # accumulation group (one start/stop for the bank)
                    for q in range(8):
                        nc.tensor.matmul(tpp[:, q * P:(q + 1) * P],
                                         lhsT=hsum[:, q * P:(q + 1) * P],
                                         rhs=ident[:], is_transpose=True,
                                         start=(q == 0), stop=(q == 7))
                    for q in range(8):
                        nc.vector.tensor_copy(
                            htgt[q][:, bi * P:(bi + 1) * P],
                            tpp[:, q * P:(q + 1) * P])

                def wp1_group(g):
                    glo, ghi = GROUPS[g]
                    gw = (ghi - glo) * P
                    tps = ps.tile([P, gw], F32, tag="tps", bufs=1)
                    ts = sp.tile([P, gw], BF16, tag="ts", bufs=1,
                                 name=f"ts{g}")
                    for o2 in range(8):
                        for k in range(8):
                            nc.tensor.matmul(
                                tps[:],
                                lhsT=wp1t[k][:, o2 * P:(o2 + 1) * P],
                                rhs=htgt[k][:],
                                start=(k == 0), stop=(k == 7))
                        if g < 2:
                            nc.scalar.activation(
                                ts[:], tps[:],
                                mybir.ActivationFunctionType.Tanh,
                                bias=bp1c[:, o2:o2 + 1],
                                accum_out=accT[:, o2 * 4 + g:o2 * 4 + g + 1])
                        else:
                            # valid nodes end at col 16 (node 1040) on the
                            # masked core; split for the mean mask
                            nc.scalar.activation(
                                ts[:, 0:16], tps[:, 0:16],
                                mybir.ActivationFunctionType.Tanh,
                                bias=bp1c[:, o2:o2 + 1],
                                accum_out=accT[:, o2 * 4 + 2:o2 * 4 + 3])
                            nc.scalar.activation(
                                ts[:, 16:gw], tps[:, 16:gw],
                                mybir.ActivationFunctionType.Tanh,
                                bias=bp1c[:, o2:o2 + 1],
                                accum_out=accT[:, o2 * 4 + 3:o2 * 4 + 4])

                prep_io(0)
                for b in range(BPC + 3):
                    if b < BPC:
                        prep_compute(b)
                    if 2 <= b <= BPC + 1:
                        stage2a(b - 2)
                    if 3 <= b <= BPC + 2:
                        stage2b(b - 3)
                        if (b - 3) in (3, 7, 9):
                            wp1_group((b - 3) // 4)
                    if b < BPC:
                        prep_alpha(b)
                    if 1 <= b <= BPC:
                        agg_matmuls(b - 1)
                    if b + 1 < BPC:
                        prep_io(b + 1)
                    if 1 <= b <= BPC:
                        agg_copies(b - 1)

            # ================= tail: softmax + blend =================
            with tc.tile_pool(name="pTsb", bufs=1) as sp, \
                 tc.tile_pool(name="pTps", bufs=1, space="PSUM") as ps:
                # WP2 loads overlap the collective latency
                wp2t = [sp.tile([P, OUT_C], BF16, tag=f"wp2_{k}",
                                name=f"wp2_{k}") for k in range(8)]
                for k in range(8):
                    nc.sync.dma_start(wp2t[k][:],
                                      WP2.ap()[k * P:(k + 1) * P, :])
                for o2 in range(8):
                    t01 = sp.tile([P, 1], F32, tag="t01", bufs=2)
                    nc.vector.tensor_tensor(
                        out=t01[:], in0=accT[:, 4 * o2:4 * o2 + 1],
                        in1=accT[:, 4 * o2 + 1:4 * o2 + 2], op=AddOp)
                    t02 = sp.tile([P, 1], F32, tag="t02", bufs=2)
                    nc.vector.tensor_tensor(
                        out=t02[:], in0=t01[:],
                        in1=accT[:, 4 * o2 + 2:4 * o2 + 3], op=AddOp)
                    nc.vector.scalar_tensor_tensor(
                        out=tbar[:, o2:o2 + 1],
                        in0=accT[:, 4 * o2 + 3:4 * o2 + 4],
                        scalar=msk_col[:, 0:1], in1=t02[:],
                        op0=MulOp, op1=AddOp)
                arview = [[1, P], [P, 8]]
                nc.sync.dma_start(bass.AP(ARIN, 0, arview), tbar[:])
                nc.gpsimd.collective_compute(
                    "AllReduce", AddOp,
                    replica_groups=[list(range(NCORES))],
                    ins=[ARIN.ap().opt()], outs=[AROUT.ap().opt()])
                tbm = sp.tile([P, 8], F32, tag="tbm")
                nc.sync.dma_start(tbm[:], bass.AP(AROUT, 0, arview))
                tbn = sp.tile([P, 8], BF16, tag="tbn")
                nc.vector.tensor_scalar_mul(tbn[:], tbm[:], 1.0 / N)
                pw = ps.tile([1, OUT_C], F32, tag="pw")
                for k in range(8):
                    nc.tensor.matmul(pw[:, 0:512], lhsT=tbn[:, k:k + 1],
                                     rhs=wp2t[k][:, 0:512], start=(k == 0),
                                     stop=(k == 7))
                    nc.tensor.matmul(pw[:, 512:1024], lhsT=tbn[:, k:k + 1],
                                     rhs=wp2t[k][:, 512:1024], start=(k == 0),
                                     stop=(k == 7))
                et = sp.tile([1, OUT_C], F32, tag="et")
                esum = sp.tile([1, 1], F32, tag="esum")
                nc.scalar.activation(et[:], pw[:],
                                     mybir.ActivationFunctionType.Exp,
                                     accum_out=esum[:])
                rs = sp.tile([1, 1], F32, tag="rs")
                nc.vector.reciprocal(rs[:], esum[:])
                att1 = sp.tile([1, OUT_C], BF16, tag="att1")
                nc.vector.tensor_scalar_mul(att1[:], et[:], rs[:, 0:1])
                # broadcast att across partitions via rank-1 matmul
                ones1r = sp.tile([1, P], BF16, tag="ones1r")
                nc.vector.memset(ones1r[:], 1.0)
                attps = ps.tile([P, OUT_C], F32, tag="attps")
                for hh in range(2):
                    nc.tensor.matmul(attps[:, hh * 512:(hh + 1) * 512],
                                     lhsT=ones1r[:],
                                     rhs=att1[:, hh * 512:(hh + 1) * 512])
                attbh = sp.tile([P, OUT_C], BF16, tag="attbh")
                nc.vector.tensor_copy(attbh[:], attps[:])
                for b in range(BPC):
                    hcols = slice(b * OUT_C, (b + 1) * OUT_C)
                    d = sp.tile([P, OUT_C], BF16, tag="bd", bufs=2)
                    nc.vector.tensor_tensor(out=d[:], in0=h1st[:, hcols],
                                            in1=h2st[:, hcols], op=SubOp)
                    m = sp.tile([P, OUT_C], BF16, tag="bm", bufs=2)
                    nc.vector.tensor_tensor(out=m[:], in0=d[:], in1=attbh[:],
                                            op=MulOp)
                    o = sp.tile([P, OUT_C], F32, tag="bo", bufs=2)
                    nc.vector.tensor_tensor(out=o[:], in0=m[:],
                                            in1=h2st[:, hcols], op=AddOp)
                    nc.sync.dma_start(OUT.ap()[b * P:(b + 1) * P, :], o[:])

    nc.compile()
    return nc


_PROG_CACHE = {}


def _ensure_trace_support():
    """Install the missing antenv.axon_hooks NTFF shim so trace=True works."""
    import types
    try:
        from antenv import axon_hooks  # noqa: F401
        return True
    except ImportError:
        pass
    try:
        import antenv
        if "/root/.axon_site" not in sys.path:
            sys.path.append("/root/.axon_site")
        from trn_agent_boot.trn_boot import _ntff_profile_via_ctypes
        hook = _ntff_profile_via_ctypes("/opt/axon/libaxon_pjrt.so")
        if hook is None:
            return False
        mod = types.ModuleType("antenv.axon_hooks")
        mod._hook = hook
        mod.get_axon_ntff_profile_hook = lambda: mod._hook
        mod.set_axon_ntff_profile_hook = lambda h: setattr(mod, "_hook", h)
        sys.modules["antenv.axon_hooks"] = mod
        antenv.axon_hooks = mod
        bass_utils.upload_artifacts = lambda t: str(t)
        return True
    except Exception as e:  # noqa: BLE001
        print("trace support unavailable:", e)
        return False


def _get_program(K):
    if K not in _PROG_CACHE:
        _PROG_CACHE[K] = _build_program(K)
    return _PROG_CACHE[K]


def _run(inputs, trace=False, tmpdir=None):
    x = np.asarray(inputs["x"], np.float32)
    edge_index = np.asarray(inputs["edge_index"])
    K, src32, em, emt, dbidx = _host_prep(edge_index)
    nc = _get_program(K)

    xpad = np.zeros((NPAD, IN_C + 4), np.float32)
    xpad[:N, :IN_C] = x
    X516 = np.ascontiguousarray(xpad).astype(ml_dtypes.bfloat16)
    XT = np.ascontiguousarray(xpad[:, :IN_C].T).astype(ml_dtypes.bfloat16)
    W1f = np.asarray(inputs["W1"], np.float32)
    W2f = np.asarray(inputs["W2"], np.float32)
    WT12 = np.ascontiguousarray(
        np.concatenate([W1f.T, W2f.T], axis=0)).astype(ml_dtypes.bfloat16)
    W12 = np.ascontiguousarray(
        np.concatenate([W1f, W2f], axis=0)).astype(ml_dtypes.bfloat16)
    A4 = np.ascontiguousarray(np.stack(
        [np.asarray(inputs["a_src1"], np.float32),
         np.asarray(inputs["a_dst1"], np.float32),
         np.asarray(inputs["a_src2"], np.float32),
         np.asarray(inputs["a_dst2"], np.float32)],
        axis=1)).astype(ml_dtypes.bfloat16)
    B12 = np.ascontiguousarray(np.stack(
        [np.asarray(inputs["b1"], np.float32),
         np.asarray(inputs["b2"], np.float32)], axis=0))
    base = {
        "X516": X516, "XT": XT, "WT12": WT12, "A4": A4, "W12": W12,
        "WP1": np.ascontiguousarray(
            np.asarray(inputs["Wp1"], np.float32)).astype(ml_dtypes.bfloat16),
        "WP2": np.ascontiguousarray(
            np.asarray(inputs["Wp2"], np.float32)).astype(ml_dtypes.bfloat16),
        "B12": B12,
        "BP1C": np.ascontiguousarray(
            np.asarray(inputs["bp1"], np.float32).reshape(8, P).T),
        "PRA": np.asarray(inputs["prelu_a"], np.float32).reshape(1, 1),
    }
    in_maps = []
    for c in range(NCORES):
        m = dict(base)
        m["MSK"] = np.array([[0.0 if c == NCORES - 1 else 1.0]], np.float32)
        m["EMB"] = np.ascontiguousarray(em[c]).astype(ml_dtypes.bfloat16)
        m["EMTB"] = np.ascontiguousarray(emt[c]).astype(ml_dtypes.bfloat16)
        m["SRC32"] = np.ascontiguousarray(src32[c])
        m["DBIDX"] = np.ascontiguousarray(dbidx[c])
        in_maps.append(m)

    if trace:
        trace = _ensure_trace_support()
    res = bass_utils.run_bass_kernel_spmd(
        nc, in_maps, core_ids=list(range(NCORES)), trace=trace,
        tmpdir=tmpdir)
    out = np.concatenate([res.results[c]["OUT"] for c in range(NCORES)],
                         axis=0)[:N]
    return out, res.exec_time_ns


def kernel(**inputs):
    out, _ = _run(inputs, trace=False)
    return out
